# revision 43
# baseline (speedup 1.0000x reference)
"""Trainium2 Bass kernel for nn_AttentionBlock (GroupNorm32 + 4-head self
attention over 64x64 spatial + output projection + residual).

Sharding over 8 NeuronCores: core = (sample s, head-group hg) with
s = core // 2, hg = core % 2 selecting global heads {2*hg, 2*hg+1}.

Per-core pipeline:
  groupnorm (bf16 x, f32 stats, fp8 xn) -> QKV via fp8 DoubleRow matmuls ->
  q,k bias-added to fp8e4 and DMA-remapped into DoubleRow layout [33, 2, HW]
  (d-halves stacked in the free dim; partition 32 carries a constant row
  contributing +24 to every score, which both shifts exp into a safe range
  and guards the fast-exp clamp) -> scores via fp8 DoubleRow matmuls
  (0.5 cyc/row, 2 k-tiles per instruction) -> exp alternating between ACT
  (native Exp -> fp8) and DVE (fast-exp bit trick rint(max(s'/ln2, 0)) ->
  uint8 == fp8e4 bytes); GPSIMD cannot touch PSUM so it instead handles the
  SBUF-side normalize (partition_broadcast of 1/sums + multiply) -> AV via
  fp8 DoubleRow with a trailing ones column producing softmax denominators
  in PSUM row 64 -> per-(ic,h) reciprocal on DVE (PE-broadcast fast path
  for the last i-chunk to shorten the tail) -> per-head f32r projection
  accumulated over heads in a spool PSUM slot -> [128, 2, HW] partial
  output per core.  AV matmuls trail their exp by AV_LAG chunks so the
  in-order PE never blocks score production on an unfinished exp; proj for
  ic is emitted mid-way through ic+1 so its stn dependency is long ready.
  Host adds residual, proj bias, and the constant v-bias correction
  proj_w[:,shard] @ bv.
"""
import numpy as np

NUM_GROUPS = 32
EPS = 1e-5
B, C, Hs, Ws = 4, 256, 64, 64
NHEADS = 4
D = C // NHEADS          # 64
HW = Hs * Ws             # 4096
N_CORES = 8
NI = 512                 # i-chunk (query positions per chunk)
NIC = HW // NI           # 8 i-chunks
NJP = HW // 256          # 16 j-tile-pairs per head
LN2_INV = float(1.0 / np.log(2.0))
AUG = 24.0               # constant added to every raw score via aug row
# ACT-path bias so both exp paths encode p = 2^{-56.5/8} * e^{s_raw/8}
C_LN = float(-AUG * 0.125 + (AUG / np.log(2.0) - 56.5) / 8.0 * np.log(2.0))

_cache = {}
AV_LAG = 3

# exp-chunk engine schedule: ACT faster per row than DVE (0.833 vs 1.042)
_EXP_PAT = ("A", "D")


def _build_module():
    from contextlib import ExitStack
    import concourse.bass as bass
    import concourse.tile as tile
    from concourse import bacc, mybir

    f32 = mybir.dt.float32
    f32r = mybir.dt.float32r
    bf16 = mybir.dt.bfloat16
    fp8 = mybir.dt.float8e4
    u8 = mybir.dt.uint8
    ALU = mybir.AluOpType
    ACTF = mybir.ActivationFunctionType
    DR = mybir.MatmulPerfMode.DoubleRow
    ts = bass.ts

    nc = bacc.Bacc("TRN2", target_bir_lowering=False, debug=False,
                   num_devices=N_CORES)

    def din(name, shape, dt=f32):
        return nc.dram_tensor(name, shape, dt, kind="ExternalInput").ap()

    x_d = din("x_s", [2, 128, HW], bf16)        # c-half, c-part, i
    wqb_d = din("wqb", [128, 2, 128], fp8)      # c-part, c-half, d2h
    wkb_d = din("wkb", [128, 2, 128], fp8)
    wvb_d = din("wvb", [128, 2, 128], fp8)
    wp_d = din("wp", [64, 512])                 # d, (h, oc)*128  (f32)
    bq_d = din("bq", [128, 1])
    bk_d = din("bk", [128, 1])
    gnsc_d = din("gnsc", [128, 2])
    gnbi_d = din("gnbi", [128, 2])
    sel_d = din("sel", [128, 64])
    rep_d = din("rep", [32, 256])
    augq_d = din("augq", [1, 2, HW], u8)        # fp8 bytes: 6.0 / 0.0
    augk_d = din("augk", [1, 2, HW], u8)        # fp8 bytes: 4.0 / 0.0
    out_d = nc.dram_tensor("outp", [128, 2, HW], f32, kind="ExternalOutput").ap()

    with tile.TileContext(nc) as tc, ExitStack() as ctx:
        persist = ctx.enter_context(tc.tile_pool(name="persist", bufs=1))

        # ---- long-lived tiles ----
        qTr = persist.tile([128, 2, HW], fp8, tag="qTr")   # parts 0-32 h0, 64-96 h1
        kTr = persist.tile([128, 2, HW], fp8, tag="kTr")
        v_aug = persist.tile([128, NJP, 2, 2, 128], fp8, tag="vaug")
        xn8 = persist.tile([128, 2, HW], fp8, tag="xn8")
        wqt = persist.tile([128, 2, 128], fp8, tag="wqt")
        wkt = persist.tile([128, 2, 128], fp8, tag="wkt")
        wvt = persist.tile([128, 2, 128], fp8, tag="wvt")
        nc.gpsimd.dma_start(wqt[:], wqb_d)
        nc.gpsimd.dma_start(wkt[:], wkb_d)
        nc.gpsimd.dma_start(wvt[:], wvb_d)
        wpf = persist.tile([64, 512], f32, tag="wpf")
        nc.gpsimd.dma_start(wpf[:], wp_d)
        wpr = persist.tile([64, 2, 2, 128], f32r, tag="wpr")
        nc.gpsimd.tensor_copy(wpr[:], wpf[:].rearrange("p (h o f) -> p h o f", h=2, o=2))
        bq = persist.tile([128, 1], f32, tag="bq")
        nc.gpsimd.dma_start(bq[:], bq_d)
        bk = persist.tile([128, 1], f32, tag="bk")
        nc.gpsimd.dma_start(bk[:], bk_d)
        expb = persist.tile([128, 1], f32, tag="expb")
        nc.vector.memset(expb[:], C_LN)
        onesf = persist.tile([65, 64], f32, tag="onesf")
        nc.gpsimd.memset(onesf[:], 1.0)
        ones64 = persist.tile([65, 64], f32r, tag="ones64")
        nc.gpsimd.tensor_copy(ones64[:], onesf[:])
        # aug rows (constant fp8 bytes) into partitions 32 and 96
        for base in (32, 96):
            nc.sync.dma_start(qTr[base:base + 1, :, :].bitcast(u8), augq_d)
            nc.sync.dma_start(kTr[base:base + 1, :, :].bitcast(u8), augk_d)
        # ones column of v_aug
        nc.vector.memset(v_aug[:, :, :, :, 64:65], 1.0)

        # ---- load x + groupnorm stats ----
        with tc.tile_pool(name="early", bufs=1) as early, \
             tc.tile_pool(name="gnps", bufs=1, space="PSUM") as gnps:
            xt = early.tile([128, 2, HW], bf16, tag="xt")
            bnout = early.tile([128, 2, 8, 6], f32, tag="bnout")
            for c in (0, 1):
                for q in range(4):
                    nc.sync.dma_start(xt[:, c, ts(q, HW // 4)],
                                      x_d[c, :, ts(q, HW // 4)])
                    for u in (0, 1):
                        nc.vector.bn_stats(bnout[:, c, 2 * q + u, :],
                                           xt[:, c, ts(2 * q + u, 512)])
            gnsc = early.tile([128, 2], f32, tag="gnsc")
            gnbi = early.tile([128, 2], f32, tag="gnbi")
            nc.gpsimd.dma_start(gnsc[:], gnsc_d)
            nc.gpsimd.dma_start(gnbi[:], gnbi_d)
            sel = early.tile([128, 64], f32, tag="sel")
            nc.gpsimd.dma_start(sel[:], sel_d)
            rep = early.tile([32, 256], f32, tag="rep")
            nc.gpsimd.dma_start(rep[:], rep_d)

            stats = [early.tile([128, 2], f32, tag=f"st{c}", name=f"st{c}")
                     for c in (0, 1)]
            for c in (0, 1):
                nc.vector.bn_aggr(stats[c][:], bnout[:, c, :, :])  # (mean, var)
                mt = early.tile([128, 1], f32, tag="mt", name="mt")
                nc.vector.tensor_tensor(out=mt[:], in0=stats[c][:, 0:1],
                                        in1=stats[c][:, 0:1], op=ALU.mult)
                nc.vector.tensor_tensor(out=stats[c][:, 1:2],
                                        in0=stats[c][:, 1:2], in1=mt[:],
                                        op=ALU.add)
            gs_ps = gnps.tile([32, 2], f32, tag="gs")
            nc.tensor.matmul(gs_ps[:], lhsT=sel[:, 0:32], rhs=stats[0][:],
                             start=True, stop=False)
            nc.tensor.matmul(gs_ps[:], lhsT=sel[:, 32:64], rhs=stats[1][:],
                             start=False, stop=True)
            gs = early.tile([32, 2], f32, tag="gs_sb")
            nc.vector.tensor_copy(gs[:], gs_ps[:])
            rg = early.tile([32, 2], f32, tag="rg")  # col0 rstd, col1 mean
            msq = early.tile([32, 2], f32, tag="msq")
            nc.vector.tensor_copy(rg[:, 1:2], gs[:, 0:1])
            nc.vector.tensor_tensor(out=msq[:, 0:1], in0=gs[:, 0:1],
                                    in1=gs[:, 0:1], op=ALU.mult)
            nc.vector.tensor_tensor(out=msq[:, 1:2], in0=gs[:, 1:2],
                                    in1=msq[:, 0:1], op=ALU.subtract)
            eps_t = early.tile([32, 1], f32, tag="eps")
            nc.vector.memset(eps_t[:], EPS)
            sd = early.tile([32, 1], f32, tag="sd")
            nc.scalar.activation(sd[:], msq[:, 1:2], ACTF.Sqrt, bias=eps_t[:])
            nc.vector.reciprocal(rg[:, 0:1], sd[:])
            AB = [early.tile([128, 2], f32, tag=f"ab{c}", name=f"ab{c}")
                  for c in (0, 1)]
            for c in (0, 1):
                ab_ps = gnps.tile([128, 2], f32, tag="ab", name="ab")
                nc.tensor.matmul(ab_ps[:], lhsT=rep[:, ts(c, 128)], rhs=rg[:],
                                 start=True, stop=True)
                nc.vector.tensor_tensor(out=AB[c][:, 0:1], in0=ab_ps[:, 0:1],
                                        in1=gnsc[:, c:c + 1], op=ALU.mult)
                tmp = early.tile([128, 1], f32, tag=f"tmp{c}", name=f"tmp{c}")
                nc.vector.tensor_tensor(out=tmp[:], in0=ab_ps[:, 1:2],
                                        in1=AB[c][:, 0:1], op=ALU.mult)
                nc.vector.tensor_tensor(out=AB[c][:, 1:2], in0=gnbi[:, c:c + 1],
                                        in1=tmp[:], op=ALU.subtract)
            # xn = A*x + B -> fp8, split Pool / ACT(Identity)
            for t in range(8):
                for c in (0, 1):
                    if t % 2 == 0:
                        nc.gpsimd.tensor_scalar(out=xn8[:, c, ts(t, NI)],
                                                in0=xt[:, c, ts(t, NI)],
                                                scalar1=AB[c][:, 0:1],
                                                scalar2=AB[c][:, 1:2],
                                                op0=ALU.mult, op1=ALU.add)
                    else:
                        nc.scalar.activation(xn8[:, c, ts(t, NI)],
                                             xt[:, c, ts(t, NI)],
                                             ACTF.Identity,
                                             scale=AB[c][:, 0:1],
                                             bias=AB[c][:, 1:2])

        # ---- QKV ----
        with tc.tile_pool(name="qkps", bufs=3, space="PSUM") as qkps, \
             tc.tile_pool(name="vps", bufs=2, space="PSUM") as vps, \
             tc.tile_pool(name="qkst", bufs=1) as qkst:
            k8 = qkst.tile([128, HW], fp8, tag="k8")
            q8 = qkst.tile([128, HW], fp8, tag="q8")

            def qk_chunk(w_t, b_t, dst, t, eng):
                ps = qkps.tile([128, NI], f32, tag="qk", name="qk")
                nc.tensor.matmul(ps[:], lhsT=w_t[:],
                                 rhs=xn8[:, :, ts(t, NI)],
                                 start=True, stop=True, perf_mode=DR)
                if eng == "D":
                    nc.vector.tensor_scalar(out=dst[:, ts(t, NI)],
                                            in0=ps[:], scalar1=b_t[:],
                                            scalar2=None, op0=ALU.add)
                else:
                    nc.scalar.activation(dst[:, ts(t, NI)], ps[:],
                                         ACTF.Identity, bias=b_t[:])

            def remap_chunk(src_t, dst_t, t):
                # partition regroup into DoubleRow layout for columns of chunk t
                for hh in (0, 1):
                    for sl in (0, 1):
                        nc.sync.dma_start(
                            dst_t[64 * hh:64 * hh + 32, sl, ts(t, NI)],
                            src_t[64 * hh + 32 * sl:64 * hh + 32 * sl + 32,
                                  ts(t, NI)])

            def v_chunk(jp, eng):
                vp = vps.tile([128, 2, 128], f32, tag="v", name="v")
                for jt in (0, 1):
                    nc.tensor.matmul(vp[:, jt, :],
                                     lhsT=xn8[:, :, ts(2 * jp + jt, 128)],
                                     rhs=wvt[:], start=True, stop=True,
                                     perf_mode=DR)
                # src order (jt, h*64+d) == dst order (jt, h, d)
                vv = vp[:].rearrange("p a (b c) -> p a b c", b=2)
                if eng == "D":
                    nc.vector.tensor_copy(v_aug[:, jp, :, :, 0:64], vv)
                else:
                    nc.scalar.activation(v_aug[:, jp, :, :, 0:64], vv,
                                         ACTF.Copy)

            for t in range(8):
                qk_chunk(wkt, bk, k8, t, "A" if t % 2 else "D")
            for t in range(8):
                qk_chunk(wqt, bq, q8, t, "D" if t % 2 else "A")
            for (s8, dst8) in ((k8, kTr), (q8, qTr)):
                for hh in (0, 1):
                    for sl in (0, 1):
                        nc.sync.dma_start(
                            dst8[64 * hh:64 * hh + 32, sl, :],
                            s8[64 * hh + 32 * sl:64 * hh + 32 * sl + 32, :])
            for jp in range(NJP):
                v_chunk(jp, "A" if jp % 2 else "D")

        # ---- attention + normalize + projection ----
        # Flat pipelined emission: AV matmuls trail their exp by AV_LAG
        # chunks so the in-order PE never blocks score production on an
        # unfinished exp; proj for ic is emitted mid-way through ic+1.
        expb_i = 0
        with tc.tile_pool(name="spool", bufs=3, space="PSUM") as spool, \
             tc.tile_pool(name="avpool", bufs=2, space="PSUM") as avpool, \
             tc.tile_pool(name="ptpool", bufs=6) as ptpool, \
             tc.tile_pool(name="stpool", bufs=6) as stpool, \
             tc.tile_pool(name="stnpool", bufs=4) as stnpool, \
             tc.tile_pool(name="bcsp", bufs=2) as bcsp, \
             tc.tile_pool(name="ppst", bufs=2) as ppstp, \
             tc.tile_pool(name="sumsp", bufs=4) as sumsp:
            st_tiles = {}
            stn = {}
            pend = []          # (gchunk, (ic,h,jp), av, lhsT, pt)

            def drain_one():
                _, (ic0, h0, jp0), av0, lhsT0, pt0 = pend.pop(0)
                nc.tensor.matmul(av0[:], lhsT=lhsT0, rhs=pt0.bitcast(fp8),
                                 start=(jp0 == 0), stop=(jp0 == NJP - 1),
                                 perf_mode=DR)
                if jp0 == NJP - 1:
                    st = stpool.tile([65, NI], f32r, tag="st",
                                     name=f"st{ic0}{h0}")
                    nc.scalar.activation(st[:], av0[:], ACTF.Copy)
                    st_tiles[(ic0, h0)] = st
                    sn = stnpool.tile([64, NI], f32r, tag="sn",
                                      name=f"sn{ic0}{h0}")
                    if ic0 == NIC - 1:
                        # tail path: no DMA hop; recip at partition 64,
                        # PE broadcast, DVE multiply
                        rr65 = stpool.tile([65, NI], f32r, tag="st",
                                           name=f"rr65{ic0}{h0}")
                        with nc.allow_low_precision(reason="recip rows"):
                            nc.vector.reciprocal(rr65[64:65, :],
                                                 st[64:65, :])
                        bcp = avpool.tile([64, NI], f32, tag="av",
                                          name=f"bcp{ic0}{h0}")
                        nc.tensor.matmul(bcp[:], lhsT=ones64[64:65, :],
                                         rhs=rr65[64:65, :],
                                         start=True, stop=True)
                        nc.vector.tensor_tensor(out=sn[:], in0=st[0:64, :],
                                                in1=bcp[:], op=ALU.mult)
                    else:
                        # recip row (DVE) -> broadcast + mult (Pool)
                        sr = sumsp.tile([1, NI], f32, tag="sr", name="sr")
                        nc.sync.dma_start(sr[:], st[64:65, :].bitcast(f32))
                        rr = sumsp.tile([1, NI], f32, tag="rr", name="rr")
                        nc.vector.reciprocal(rr[:], sr[:])
                        bcs = bcsp.tile([64, NI], f32, tag="bc", name="bc")
                        nc.gpsimd.partition_broadcast(bcs[:], rr[:])
                        nc.gpsimd.tensor_tensor(out=sn[:], in0=st[0:64, :],
                                                in1=bcs[:], op=ALU.mult)
                    stn[(ic0, h0)] = sn

            def emit_proj(ic0):
                pp = spool.tile([128, 2, NI], f32, tag="sp", name=f"pp{ic0}")
                for oc in (0, 1):
                    for h2 in (0, 1):
                        nc.tensor.matmul(pp[:, oc, :],
                                         lhsT=wpr[:, h2, oc, :],
                                         rhs=stn[(ic0, h2)][:],
                                         start=(h2 == 0), stop=(h2 == 1))
                po = ppstp.tile([128, 2, NI], f32, tag="po", name="po")
                nc.scalar.activation(po[:], pp[:], ACTF.Copy)
                nc.gpsimd.dma_start(out_d[:, :, ts(ic0, NI)], po[:])

            g = 0
            for ic in range(NIC):
                for h in (0, 1):
                    hb = 64 * h
                    av = avpool.tile([65, NI], f32, tag="av", name=f"av{ic}{h}")
                    for jp in range(NJP):
                        if h == 1 and jp == NJP // 2 and ic > 0:
                            emit_proj(ic - 1)
                        sp = spool.tile([128, 2, NI], f32, tag="sp", name="sp")
                        for jt in (0, 1):
                            nc.tensor.matmul(
                                sp[:, jt, :],
                                lhsT=kTr[hb:hb + 33, :, ts(2 * jp + jt, 128)],
                                rhs=qTr[hb:hb + 33, :, ts(ic, NI)],
                                start=True, stop=True, perf_mode=DR)
                        pt = ptpool.tile([128, 2, NI], u8, tag="pt", name="pt")
                        e = _EXP_PAT[expb_i % len(_EXP_PAT)]
                        expb_i += 1
                        if e == "A":
                            nc.scalar.activation(pt[:].bitcast(fp8), sp[:],
                                                 ACTF.Exp, scale=0.125,
                                                 bias=expb[:])
                        else:
                            nc.vector.tensor_scalar(out=pt[:], in0=sp[:],
                                                    scalar1=LN2_INV,
                                                    scalar2=0.0,
                                                    op0=ALU.mult, op1=ALU.max)
                        pend.append((g, (ic, h, jp), av,
                                     v_aug[:, jp, :, h, 0:65], pt[:]))
                        while pend and pend[0][0] <= g - AV_LAG:
                            drain_one()
                        g += 1
            while pend:
                drain_one()
            emit_proj(NIC - 1)

    nc.compile()
    return nc


def _host_inputs(x, gn_scale, gn_bias, qkv_w, qkv_b, proj_w):
    import ml_dtypes
    x = np.ascontiguousarray(np.asarray(x, dtype=np.float32))
    gn_scale = np.asarray(gn_scale, dtype=np.float32)
    gn_bias = np.asarray(gn_bias, dtype=np.float32)
    qkv_w = np.asarray(qkv_w, dtype=np.float32)
    qkv_b = np.asarray(qkv_b, dtype=np.float32)
    proj_w = np.asarray(proj_w, dtype=np.float32)

    sel = np.zeros((128, 64), np.float32)
    rep = np.zeros((32, 256), np.float32)
    for p in range(128):
        sel[p, p // 8] = 1.0 / 8
        sel[p, 32 + 16 + p // 8] = 1.0 / 8
        rep[p // 8, p] = 1.0
        rep[16 + p // 8, 128 + p] = 1.0

    def aug_row(v):  # [1, 2, HW] fp8 bytes: slot0 = v, slot1 = 0
        a = np.zeros((1, 2, HW), dtype=ml_dtypes.float8_e4m3fn)
        a[0, 0, :] = v
        return np.ascontiguousarray(a.view(np.uint8))

    augq = aug_row(6.0)
    augk = aug_row(4.0)

    def wsel(W, rows):  # [256 c, sel 128 d2h] -> [128 cpart, 2 chalf, 128] fp8
        Wt = W[rows].T  # [256 c, 128]
        return np.ascontiguousarray(
            Wt.reshape(2, 128, 128).transpose(1, 0, 2)
            .astype(ml_dtypes.float8_e4m3fn))

    in_maps = []
    corrs = []
    for core in range(N_CORES):
        s, hg = core // 2, core % 2
        rows = np.r_[2 * hg * D:(2 * hg + 1) * D,
                     (2 * hg + 1) * D:(2 * hg + 2) * D]
        wq = wsel(qkv_w[0 * C:1 * C], rows)
        wk = wsel(qkv_w[1 * C:2 * C], rows)
        wv = wsel(qkv_w[2 * C:3 * C], rows)
        # wp[d, h, oc, :] = proj_w[oc*128:(oc+1)*128, rows[h*64+d]]
        wp = np.zeros((64, 2, 2, 128), np.float32)
        for h in (0, 1):
            block = proj_w[:, rows[h * 64:(h + 1) * 64]]  # [256 oc, 64 d]
            wp[:, h, 0, :] = block[0:128].T
            wp[:, h, 1, :] = block[128:256].T
        bqv = qkv_b[rows].reshape(128, 1)
        bkv = qkv_b[C + rows].reshape(128, 1)
        bv = qkv_b[2 * C + rows]
        corrs.append(proj_w[:, rows] @ bv)
        in_maps.append({
            "x_s": np.ascontiguousarray(x[s].reshape(2, 128, HW).astype(ml_dtypes.bfloat16)),
            "wqb": wq, "wkb": wk, "wvb": wv,
            "wp": np.ascontiguousarray(wp.reshape(64, 512)),
            "bq": np.ascontiguousarray(bqv), "bk": np.ascontiguousarray(bkv),
            "gnsc": np.ascontiguousarray(gn_scale.reshape(2, 128).T),
            "gnbi": np.ascontiguousarray(gn_bias.reshape(2, 128).T),
            "sel": sel, "rep": rep, "augq": augq, "augk": augk,
        })
    return x, in_maps, corrs


def kernel(x, gn_scale, gn_bias, qkv_w, qkv_b, proj_w, proj_b, _trace=False):
    from concourse import bass_utils

    if "nc" not in _cache:
        _cache["nc"] = _build_module()
    nc = _cache["nc"]

    x, in_maps, corrs = _host_inputs(x, gn_scale, gn_bias, qkv_w, qkv_b, proj_w)
    proj_b = np.asarray(proj_b, dtype=np.float32)

    res = bass_utils.run_bass_kernel_spmd(
        nc, in_maps, core_ids=list(range(N_CORES)), trace=_trace)
    _cache["last_result"] = res

    out = np.empty((B, C, Hs, Ws), np.float32)
    for s in range(B):
        acc = x[s].reshape(C, HW).copy()
        # outp [128 part, 2 oc, HW]: channel oc*128+p = outp[p, oc, :]
        for cr in (2 * s, 2 * s + 1):
            o = res.results[cr]["outp"]
            acc += o.transpose(1, 0, 2).reshape(C, HW)
        acc += (proj_b + corrs[2 * s] + corrs[2 * s + 1])[:, None]
        out[s] = acc.reshape(C, Hs, Ws)
    return out


# revision 45
# speedup vs baseline: 1.0220x; 1.0220x over previous
"""Trainium2 Bass kernel for nn_AttentionBlock (GroupNorm32 + 4-head self
attention over 64x64 spatial + output projection + residual).

Sharding over 8 NeuronCores: core = (sample s, head-group hg) with
s = core // 2, hg = core % 2 selecting global heads {2*hg, 2*hg+1}.

Per-core pipeline:
  groupnorm (bf16 x, f32 stats, fp8 xn) -> QKV via fp8 DoubleRow matmuls ->
  q,k bias-added to fp8e4 and DMA-remapped into DoubleRow layout [33, 2, HW]
  (d-halves stacked in the free dim; partition 32 carries a constant row
  contributing +24 to every score, which both shifts exp into a safe range
  and guards the fast-exp clamp) -> scores via fp8 DoubleRow matmuls
  (0.5 cyc/row, 2 k-tiles per instruction) -> exp alternating between ACT
  (native Exp -> fp8) and DVE (fast-exp bit trick rint(max(s'/ln2, 0)) ->
  uint8 == fp8e4 bytes); GPSIMD cannot touch PSUM so it instead handles the
  SBUF-side normalize (partition_broadcast of 1/sums + multiply) -> AV via
  fp8 DoubleRow with a trailing ones column producing softmax denominators
  in PSUM row 64 -> per-(ic,h) reciprocal on DVE (PE-broadcast fast path
  for the last i-chunk to shorten the tail) -> per-head f32r projection
  accumulated over heads in a spool PSUM slot -> [128, 2, HW] partial
  output per core.  AV matmuls trail their exp by AV_LAG chunks so the
  in-order PE never blocks score production on an unfinished exp; proj for
  ic is emitted mid-way through ic+1 so its stn dependency is long ready.
  Host adds residual, proj bias, and the constant v-bias correction
  proj_w[:,shard] @ bv.
"""
import numpy as np

NUM_GROUPS = 32
EPS = 1e-5
B, C, Hs, Ws = 4, 256, 64, 64
NHEADS = 4
D = C // NHEADS          # 64
HW = Hs * Ws             # 4096
N_CORES = 8
NI = 512                 # i-chunk (query positions per chunk)
NIC = HW // NI           # 8 i-chunks
NJP = HW // 256          # 16 j-tile-pairs per head
LN2_INV = float(1.0 / np.log(2.0))
AUG = 24.0               # constant added to every raw score via aug row
# ACT-path bias so both exp paths encode p = 2^{-56.5/8} * e^{s_raw/8}
C_LN = float(-AUG * 0.125 + (AUG / np.log(2.0) - 56.5) / 8.0 * np.log(2.0))

_cache = {}
AV_LAG = 3
PROJ_JP = NJP // 2

# exp-chunk engine schedule: ACT faster per row than DVE (0.833 vs 1.042)
_EXP_PAT = ("D", "A")


def _build_module():
    from contextlib import ExitStack
    import concourse.bass as bass
    import concourse.tile as tile
    from concourse import bacc, mybir

    f32 = mybir.dt.float32
    f32r = mybir.dt.float32r
    bf16 = mybir.dt.bfloat16
    fp8 = mybir.dt.float8e4
    u8 = mybir.dt.uint8
    ALU = mybir.AluOpType
    ACTF = mybir.ActivationFunctionType
    DR = mybir.MatmulPerfMode.DoubleRow
    ts = bass.ts

    nc = bacc.Bacc("TRN2", target_bir_lowering=False, debug=False,
                   num_devices=N_CORES)

    def din(name, shape, dt=f32):
        return nc.dram_tensor(name, shape, dt, kind="ExternalInput").ap()

    x_d = din("x_s", [2, 128, HW], bf16)        # c-half, c-part, i
    wqb_d = din("wqb", [128, 2, 128], fp8)      # c-part, c-half, d2h
    wkb_d = din("wkb", [128, 2, 128], fp8)
    wvb_d = din("wvb", [128, 2, 128], fp8)
    wp_d = din("wp", [64, 512])                 # d, (h, oc)*128  (f32)
    bq_d = din("bq", [128, 1])
    bk_d = din("bk", [128, 1])
    gnsc_d = din("gnsc", [128, 2])
    gnbi_d = din("gnbi", [128, 2])
    sel_d = din("sel", [128, 64])
    rep_d = din("rep", [32, 256])
    augq_d = din("augq", [1, 2, HW], u8)        # fp8 bytes: 6.0 / 0.0
    augk_d = din("augk", [1, 2, HW], u8)        # fp8 bytes: 4.0 / 0.0
    out_d = nc.dram_tensor("outp", [128, 2, HW], f32, kind="ExternalOutput").ap()

    with tile.TileContext(nc) as tc, ExitStack() as ctx:
        persist = ctx.enter_context(tc.tile_pool(name="persist", bufs=1))

        # ---- long-lived tiles ----
        qTr = persist.tile([128, 2, HW], fp8, tag="qTr")   # parts 0-32 h0, 64-96 h1
        kTr = persist.tile([128, 2, HW], fp8, tag="kTr")
        v_aug = persist.tile([128, NJP, 2, 2, 128], fp8, tag="vaug")
        xn8 = persist.tile([128, 2, HW], fp8, tag="xn8")
        wqt = persist.tile([128, 2, 128], fp8, tag="wqt")
        wkt = persist.tile([128, 2, 128], fp8, tag="wkt")
        wvt = persist.tile([128, 2, 128], fp8, tag="wvt")
        nc.gpsimd.dma_start(wqt[:], wqb_d)
        nc.gpsimd.dma_start(wkt[:], wkb_d)
        nc.gpsimd.dma_start(wvt[:], wvb_d)
        wpf = persist.tile([64, 512], f32, tag="wpf")
        nc.gpsimd.dma_start(wpf[:], wp_d)
        wpr = persist.tile([64, 2, 2, 128], f32r, tag="wpr")
        nc.gpsimd.tensor_copy(wpr[:], wpf[:].rearrange("p (h o f) -> p h o f", h=2, o=2))
        bq = persist.tile([128, 1], f32, tag="bq")
        nc.gpsimd.dma_start(bq[:], bq_d)
        bk = persist.tile([128, 1], f32, tag="bk")
        nc.gpsimd.dma_start(bk[:], bk_d)
        expb = persist.tile([128, 1], f32, tag="expb")
        nc.vector.memset(expb[:], C_LN)
        onesf = persist.tile([65, 64], f32, tag="onesf")
        nc.gpsimd.memset(onesf[:], 1.0)
        ones64 = persist.tile([65, 64], f32r, tag="ones64")
        nc.gpsimd.tensor_copy(ones64[:], onesf[:])
        # aug rows (constant fp8 bytes) into partitions 32 and 96
        for base in (32, 96):
            nc.sync.dma_start(qTr[base:base + 1, :, :].bitcast(u8), augq_d)
            nc.sync.dma_start(kTr[base:base + 1, :, :].bitcast(u8), augk_d)
        # ones column of v_aug
        nc.vector.memset(v_aug[:, :, :, :, 64:65], 1.0)

        # ---- load x + groupnorm stats ----
        with tc.tile_pool(name="early", bufs=1) as early, \
             tc.tile_pool(name="gnps", bufs=1, space="PSUM") as gnps:
            xt = early.tile([128, 2, HW], bf16, tag="xt")
            bnout = early.tile([128, 2, 8, 6], f32, tag="bnout")
            for c in (0, 1):
                for q in range(2):
                    nc.sync.dma_start(xt[:, c, ts(q, HW // 2)],
                                      x_d[c, :, ts(q, HW // 2)])
                    for u in range(4):
                        nc.vector.bn_stats(bnout[:, c, 4 * q + u, :],
                                           xt[:, c, ts(4 * q + u, 512)])
            gnsc = early.tile([128, 2], f32, tag="gnsc")
            gnbi = early.tile([128, 2], f32, tag="gnbi")
            nc.gpsimd.dma_start(gnsc[:], gnsc_d)
            nc.gpsimd.dma_start(gnbi[:], gnbi_d)
            sel = early.tile([128, 64], f32, tag="sel")
            nc.gpsimd.dma_start(sel[:], sel_d)
            rep = early.tile([32, 256], f32, tag="rep")
            nc.gpsimd.dma_start(rep[:], rep_d)

            stats = [early.tile([128, 2], f32, tag=f"st{c}", name=f"st{c}")
                     for c in (0, 1)]
            for c in (0, 1):
                nc.vector.bn_aggr(stats[c][:], bnout[:, c, :, :])  # (mean, var)
                mt = early.tile([128, 1], f32, tag="mt", name="mt")
                nc.vector.tensor_tensor(out=mt[:], in0=stats[c][:, 0:1],
                                        in1=stats[c][:, 0:1], op=ALU.mult)
                nc.vector.tensor_tensor(out=stats[c][:, 1:2],
                                        in0=stats[c][:, 1:2], in1=mt[:],
                                        op=ALU.add)
            gs_ps = gnps.tile([32, 2], f32, tag="gs")
            nc.tensor.matmul(gs_ps[:], lhsT=sel[:, 0:32], rhs=stats[0][:],
                             start=True, stop=False)
            nc.tensor.matmul(gs_ps[:], lhsT=sel[:, 32:64], rhs=stats[1][:],
                             start=False, stop=True)
            gs = early.tile([32, 2], f32, tag="gs_sb")
            nc.vector.tensor_copy(gs[:], gs_ps[:])
            rg = early.tile([32, 2], f32, tag="rg")  # col0 rstd, col1 mean
            msq = early.tile([32, 2], f32, tag="msq")
            nc.vector.tensor_copy(rg[:, 1:2], gs[:, 0:1])
            nc.vector.tensor_tensor(out=msq[:, 0:1], in0=gs[:, 0:1],
                                    in1=gs[:, 0:1], op=ALU.mult)
            nc.vector.tensor_tensor(out=msq[:, 1:2], in0=gs[:, 1:2],
                                    in1=msq[:, 0:1], op=ALU.subtract)
            eps_t = early.tile([32, 1], f32, tag="eps")
            nc.vector.memset(eps_t[:], EPS)
            sd = early.tile([32, 1], f32, tag="sd")
            nc.scalar.activation(sd[:], msq[:, 1:2], ACTF.Sqrt, bias=eps_t[:])
            nc.vector.reciprocal(rg[:, 0:1], sd[:])
            AB = [early.tile([128, 2], f32, tag=f"ab{c}", name=f"ab{c}")
                  for c in (0, 1)]
            for c in (0, 1):
                ab_ps = gnps.tile([128, 2], f32, tag="ab", name="ab")
                nc.tensor.matmul(ab_ps[:], lhsT=rep[:, ts(c, 128)], rhs=rg[:],
                                 start=True, stop=True)
                nc.vector.tensor_tensor(out=AB[c][:, 0:1], in0=ab_ps[:, 0:1],
                                        in1=gnsc[:, c:c + 1], op=ALU.mult)
                tmp = early.tile([128, 1], f32, tag=f"tmp{c}", name=f"tmp{c}")
                nc.vector.tensor_tensor(out=tmp[:], in0=ab_ps[:, 1:2],
                                        in1=AB[c][:, 0:1], op=ALU.mult)
                nc.vector.tensor_tensor(out=AB[c][:, 1:2], in0=gnbi[:, c:c + 1],
                                        in1=tmp[:], op=ALU.subtract)
            # xn = A*x + B -> fp8, split ACT/Pool/DVE
            for t in range(8):
                for c in (0, 1):
                    e = ("A", "P", "D")[(2 * t + c) % 3]
                    if e == "A":
                        nc.scalar.activation(xn8[:, c, ts(t, NI)],
                                             xt[:, c, ts(t, NI)],
                                             ACTF.Identity,
                                             scale=AB[c][:, 0:1],
                                             bias=AB[c][:, 1:2])
                    else:
                        en = nc.gpsimd if e == "P" else nc.vector
                        en.tensor_scalar(out=xn8[:, c, ts(t, NI)],
                                         in0=xt[:, c, ts(t, NI)],
                                         scalar1=AB[c][:, 0:1],
                                         scalar2=AB[c][:, 1:2],
                                         op0=ALU.mult, op1=ALU.add)

        # ---- QKV ----
        with tc.tile_pool(name="qkps", bufs=3, space="PSUM") as qkps, \
             tc.tile_pool(name="vps", bufs=2, space="PSUM") as vps, \
             tc.tile_pool(name="qkst", bufs=1) as qkst:
            k8 = qkst.tile([128, HW], fp8, tag="k8")
            q8 = qkst.tile([128, HW], fp8, tag="q8")

            def qk_chunk(w_t, b_t, dst, t, eng):
                ps = qkps.tile([128, NI], f32, tag="qk", name="qk")
                nc.tensor.matmul(ps[:], lhsT=w_t[:],
                                 rhs=xn8[:, :, ts(t, NI)],
                                 start=True, stop=True, perf_mode=DR)
                if eng == "D":
                    nc.vector.tensor_scalar(out=dst[:, ts(t, NI)],
                                            in0=ps[:], scalar1=b_t[:],
                                            scalar2=None, op0=ALU.add)
                else:
                    nc.scalar.activation(dst[:, ts(t, NI)], ps[:],
                                         ACTF.Identity, bias=b_t[:])

            def remap_chunk(src_t, dst_t, t):
                # partition regroup into DoubleRow layout for columns of chunk t
                for hh in (0, 1):
                    for sl in (0, 1):
                        nc.sync.dma_start(
                            dst_t[64 * hh:64 * hh + 32, sl, ts(t, NI)],
                            src_t[64 * hh + 32 * sl:64 * hh + 32 * sl + 32,
                                  ts(t, NI)])

            def v_chunk(jp, eng):
                vp = vps.tile([128, 2, 128], f32, tag="v", name="v")
                for jt in (0, 1):
                    nc.tensor.matmul(vp[:, jt, :],
                                     lhsT=xn8[:, :, ts(2 * jp + jt, 128)],
                                     rhs=wvt[:], start=True, stop=True,
                                     perf_mode=DR)
                # src order (jt, h*64+d) == dst order (jt, h, d)
                vv = vp[:].rearrange("p a (b c) -> p a b c", b=2)
                if eng == "D":
                    nc.vector.tensor_copy(v_aug[:, jp, :, :, 0:64], vv)
                else:
                    nc.scalar.activation(v_aug[:, jp, :, :, 0:64], vv,
                                         ACTF.Copy)

            for t in range(8):
                qk_chunk(wkt, bk, k8, t, "A" if t % 2 else "D")
            for t in range(8):
                qk_chunk(wqt, bq, q8, t, "D" if t % 2 else "A")
            for (s8, dst8) in ((k8, kTr), (q8, qTr)):
                for hh in (0, 1):
                    for sl in (0, 1):
                        nc.sync.dma_start(
                            dst8[64 * hh:64 * hh + 32, sl, :],
                            s8[64 * hh + 32 * sl:64 * hh + 32 * sl + 32, :])
            for jp in range(NJP):
                v_chunk(jp, "A" if jp % 2 else "D")

        # ---- attention + normalize + projection ----
        # Flat pipelined emission: AV matmuls trail their exp by AV_LAG
        # chunks so the in-order PE never blocks score production on an
        # unfinished exp; proj for ic is emitted mid-way through ic+1.
        expb_i = 0
        with tc.tile_pool(name="spool", bufs=3, space="PSUM") as spool, \
             tc.tile_pool(name="avpool", bufs=2, space="PSUM") as avpool, \
             tc.tile_pool(name="ptpool", bufs=6) as ptpool, \
             tc.tile_pool(name="stpool", bufs=6) as stpool, \
             tc.tile_pool(name="stnpool", bufs=4) as stnpool, \
             tc.tile_pool(name="bcsp", bufs=2) as bcsp, \
             tc.tile_pool(name="ppst", bufs=2) as ppstp, \
             tc.tile_pool(name="sumsp", bufs=4) as sumsp:
            st_tiles = {}
            stn = {}
            pend = []          # (gchunk, (ic,h,jp), av, lhsT, pt)

            def drain_one():
                _, (ic0, h0, jp0), av0, lhsT0, pt0 = pend.pop(0)
                nc.tensor.matmul(av0[:], lhsT=lhsT0, rhs=pt0.bitcast(fp8),
                                 start=(jp0 == 0), stop=(jp0 == NJP - 1),
                                 perf_mode=DR)
                if jp0 == NJP - 1:
                    st = stpool.tile([65, NI], f32r, tag="st",
                                     name=f"st{ic0}{h0}")
                    nc.scalar.activation(st[:], av0[:], ACTF.Copy)
                    st_tiles[(ic0, h0)] = st
                    sn = stnpool.tile([64, NI], f32r, tag="sn",
                                      name=f"sn{ic0}{h0}")
                    if ic0 == NIC - 1:
                        # tail path: no DMA hop; recip at partition 64,
                        # PE broadcast, DVE multiply
                        rr65 = stpool.tile([65, NI], f32r, tag="st",
                                           name=f"rr65{ic0}{h0}")
                        with nc.allow_low_precision(reason="recip rows"):
                            nc.vector.reciprocal(rr65[64:65, :],
                                                 st[64:65, :])
                        bcp = avpool.tile([64, NI], f32, tag="av",
                                          name=f"bcp{ic0}{h0}")
                        nc.tensor.matmul(bcp[:], lhsT=ones64[64:65, :],
                                         rhs=rr65[64:65, :],
                                         start=True, stop=True)
                        nc.vector.tensor_tensor(out=sn[:], in0=st[0:64, :],
                                                in1=bcp[:], op=ALU.mult)
                    else:
                        # recip row (DVE) -> broadcast + mult (Pool)
                        sr = sumsp.tile([1, NI], f32, tag="sr", name="sr")
                        nc.sync.dma_start(sr[:], st[64:65, :].bitcast(f32))
                        rr = sumsp.tile([1, NI], f32, tag="rr", name="rr")
                        nc.vector.reciprocal(rr[:], sr[:])
                        bcs = bcsp.tile([64, NI], f32, tag="bc", name="bc")
                        nc.gpsimd.partition_broadcast(bcs[:], rr[:])
                        nc.gpsimd.tensor_tensor(out=sn[:], in0=st[0:64, :],
                                                in1=bcs[:], op=ALU.mult)
                    stn[(ic0, h0)] = sn

            def emit_proj(ic0):
                pp = spool.tile([128, 2, NI], f32, tag="sp", name=f"pp{ic0}")
                for oc in (0, 1):
                    for h2 in (0, 1):
                        nc.tensor.matmul(pp[:, oc, :],
                                         lhsT=wpr[:, h2, oc, :],
                                         rhs=stn[(ic0, h2)][:],
                                         start=(h2 == 0), stop=(h2 == 1))
                po = ppstp.tile([128, 2, NI], f32, tag="po", name="po")
                nc.scalar.activation(po[:], pp[:], ACTF.Copy)
                nc.gpsimd.dma_start(out_d[:, :, ts(ic0, NI)], po[:])

            g = 0
            for ic in range(NIC):
                for h in (0, 1):
                    hb = 64 * h
                    av = avpool.tile([65, NI], f32, tag="av", name=f"av{ic}{h}")
                    for jp in range(NJP):
                        if h == 1 and jp == PROJ_JP and ic > 0:
                            emit_proj(ic - 1)
                        sp = spool.tile([128, 2, NI], f32, tag="sp", name="sp")
                        for jt in (0, 1):
                            nc.tensor.matmul(
                                sp[:, jt, :],
                                lhsT=kTr[hb:hb + 33, :, ts(2 * jp + jt, 128)],
                                rhs=qTr[hb:hb + 33, :, ts(ic, NI)],
                                start=True, stop=True, perf_mode=DR)
                        pt = ptpool.tile([128, 2, NI], u8, tag="pt", name="pt")
                        e = _EXP_PAT[expb_i % len(_EXP_PAT)]
                        expb_i += 1
                        if e == "A":
                            nc.scalar.activation(pt[:].bitcast(fp8), sp[:],
                                                 ACTF.Exp, scale=0.125,
                                                 bias=expb[:])
                        else:
                            nc.vector.tensor_scalar(out=pt[:], in0=sp[:],
                                                    scalar1=LN2_INV,
                                                    scalar2=0.0,
                                                    op0=ALU.mult, op1=ALU.max)
                        pend.append((g, (ic, h, jp), av,
                                     v_aug[:, jp, :, h, 0:65], pt[:]))
                        while pend and pend[0][0] <= g - AV_LAG:
                            drain_one()
                        g += 1
            while pend:
                drain_one()
            emit_proj(NIC - 1)

    nc.compile()
    return nc


def _host_inputs(x, gn_scale, gn_bias, qkv_w, qkv_b, proj_w):
    import ml_dtypes
    x = np.ascontiguousarray(np.asarray(x, dtype=np.float32))
    gn_scale = np.asarray(gn_scale, dtype=np.float32)
    gn_bias = np.asarray(gn_bias, dtype=np.float32)
    qkv_w = np.asarray(qkv_w, dtype=np.float32)
    qkv_b = np.asarray(qkv_b, dtype=np.float32)
    proj_w = np.asarray(proj_w, dtype=np.float32)

    sel = np.zeros((128, 64), np.float32)
    rep = np.zeros((32, 256), np.float32)
    for p in range(128):
        sel[p, p // 8] = 1.0 / 8
        sel[p, 32 + 16 + p // 8] = 1.0 / 8
        rep[p // 8, p] = 1.0
        rep[16 + p // 8, 128 + p] = 1.0

    def aug_row(v):  # [1, 2, HW] fp8 bytes: slot0 = v, slot1 = 0
        a = np.zeros((1, 2, HW), dtype=ml_dtypes.float8_e4m3fn)
        a[0, 0, :] = v
        return np.ascontiguousarray(a.view(np.uint8))

    augq = aug_row(6.0)
    augk = aug_row(4.0)

    def wsel(W, rows):  # [256 c, sel 128 d2h] -> [128 cpart, 2 chalf, 128] fp8
        Wt = W[rows].T  # [256 c, 128]
        return np.ascontiguousarray(
            Wt.reshape(2, 128, 128).transpose(1, 0, 2)
            .astype(ml_dtypes.float8_e4m3fn))

    in_maps = []
    corrs = []
    for core in range(N_CORES):
        s, hg = core // 2, core % 2
        rows = np.r_[2 * hg * D:(2 * hg + 1) * D,
                     (2 * hg + 1) * D:(2 * hg + 2) * D]
        wq = wsel(qkv_w[0 * C:1 * C], rows)
        wk = wsel(qkv_w[1 * C:2 * C], rows)
        wv = wsel(qkv_w[2 * C:3 * C], rows)
        # wp[d, h, oc, :] = proj_w[oc*128:(oc+1)*128, rows[h*64+d]]
        wp = np.zeros((64, 2, 2, 128), np.float32)
        for h in (0, 1):
            block = proj_w[:, rows[h * 64:(h + 1) * 64]]  # [256 oc, 64 d]
            wp[:, h, 0, :] = block[0:128].T
            wp[:, h, 1, :] = block[128:256].T
        bqv = qkv_b[rows].reshape(128, 1)
        bkv = qkv_b[C + rows].reshape(128, 1)
        bv = qkv_b[2 * C + rows]
        corrs.append(proj_w[:, rows] @ bv)
        in_maps.append({
            "x_s": np.ascontiguousarray(x[s].reshape(2, 128, HW).astype(ml_dtypes.bfloat16)),
            "wqb": wq, "wkb": wk, "wvb": wv,
            "wp": np.ascontiguousarray(wp.reshape(64, 512)),
            "bq": np.ascontiguousarray(bqv), "bk": np.ascontiguousarray(bkv),
            "gnsc": np.ascontiguousarray(gn_scale.reshape(2, 128).T),
            "gnbi": np.ascontiguousarray(gn_bias.reshape(2, 128).T),
            "sel": sel, "rep": rep, "augq": augq, "augk": augk,
        })
    return x, in_maps, corrs


def kernel(x, gn_scale, gn_bias, qkv_w, qkv_b, proj_w, proj_b, _trace=False):
    from concourse import bass_utils

    if "nc" not in _cache:
        _cache["nc"] = _build_module()
    nc = _cache["nc"]

    x, in_maps, corrs = _host_inputs(x, gn_scale, gn_bias, qkv_w, qkv_b, proj_w)
    proj_b = np.asarray(proj_b, dtype=np.float32)

    res = bass_utils.run_bass_kernel_spmd(
        nc, in_maps, core_ids=list(range(N_CORES)), trace=_trace)
    _cache["last_result"] = res

    out = np.empty((B, C, Hs, Ws), np.float32)
    for s in range(B):
        acc = x[s].reshape(C, HW).copy()
        # outp [128 part, 2 oc, HW]: channel oc*128+p = outp[p, oc, :]
        for cr in (2 * s, 2 * s + 1):
            o = res.results[cr]["outp"]
            acc += o.transpose(1, 0, 2).reshape(C, HW)
        acc += (proj_b + corrs[2 * s] + corrs[2 * s + 1])[:, None]
        out[s] = acc.reshape(C, Hs, Ws)
    return out


# revision 50
# speedup vs baseline: 1.0348x; 1.0125x over previous
"""Trainium2 Bass kernel for nn_AttentionBlock (GroupNorm32 + 4-head self
attention over 64x64 spatial + output projection + residual).

Sharding over 8 NeuronCores: core = (sample s, head-group hg) with
s = core // 2, hg = core % 2 selecting global heads {2*hg, 2*hg+1}.

Per-core pipeline:
  groupnorm (bf16 x, f32 stats, fp8 xn) -> QKV via fp8 DoubleRow matmuls ->
  q,k bias-added to fp8e4 and DMA-remapped into DoubleRow layout [33, 2, HW]
  (d-halves stacked in the free dim; partition 32 carries a constant row
  contributing +24 to every score, which both shifts exp into a safe range
  and guards the fast-exp clamp) -> scores via fp8 DoubleRow matmuls
  (0.5 cyc/row, 2 k-tiles per instruction) -> exp alternating between ACT
  (native Exp -> fp8) and DVE (fast-exp bit trick rint(max(s'/ln2, 0)) ->
  uint8 == fp8e4 bytes); GPSIMD cannot touch PSUM so it instead handles the
  SBUF-side normalize (partition_broadcast of 1/sums + multiply) -> AV via
  fp8 DoubleRow with a trailing ones column producing softmax denominators
  in PSUM row 64 -> per-(ic,h) reciprocal on DVE (PE-broadcast fast path
  for the last i-chunk to shorten the tail) -> per-head f32r projection
  accumulated over heads in a spool PSUM slot -> [128, 2, HW] partial
  output per core.  AV matmuls trail their exp by AV_LAG chunks so the
  in-order PE never blocks score production on an unfinished exp; proj for
  ic is emitted mid-way through ic+1 so its stn dependency is long ready.
  Host adds residual, proj bias, and the constant v-bias correction
  proj_w[:,shard] @ bv.
"""
import numpy as np

NUM_GROUPS = 32
EPS = 1e-5
B, C, Hs, Ws = 4, 256, 64, 64
NHEADS = 4
D = C // NHEADS          # 64
HW = Hs * Ws             # 4096
N_CORES = 8
NI = 512                 # i-chunk (query positions per chunk)
NIC = HW // NI           # 8 i-chunks
NJP = HW // 256          # 16 j-tile-pairs per head
LN2_INV = float(1.0 / np.log(2.0))
AUG = 24.0               # constant added to every raw score via aug row
# ACT-path bias so both exp paths encode p = 2^{-56.5/8} * e^{s_raw/8}
C_LN = float(-AUG * 0.125 + (AUG / np.log(2.0) - 56.5) / 8.0 * np.log(2.0))

_cache = {}
AV_LAG = 3
PROJ_JP = NJP // 2

# exp-chunk engine schedule: ACT faster per row than DVE (0.833 vs 1.042)
_EXP_PAT = ("D", "A")


def _build_module():
    from contextlib import ExitStack
    import concourse.bass as bass
    import concourse.tile as tile
    from concourse import bacc, mybir

    f32 = mybir.dt.float32
    f32r = mybir.dt.float32r
    bf16 = mybir.dt.bfloat16
    fp8 = mybir.dt.float8e4
    u8 = mybir.dt.uint8
    ALU = mybir.AluOpType
    ACTF = mybir.ActivationFunctionType
    DR = mybir.MatmulPerfMode.DoubleRow
    ts = bass.ts

    nc = bacc.Bacc("TRN2", target_bir_lowering=False, debug=False,
                   num_devices=N_CORES)

    def din(name, shape, dt=f32):
        return nc.dram_tensor(name, shape, dt, kind="ExternalInput").ap()

    x_d = din("x_s", [2, 128, HW], bf16)        # c-half, c-part, i
    wqb_d = din("wqb", [128, 2, 128], fp8)      # c-part, c-half, d2h
    wkb_d = din("wkb", [128, 2, 128], fp8)
    wvb_d = din("wvb", [128, 2, 128], fp8)
    wp_d = din("wp", [64, 512])                 # d, (h, oc)*128  (f32)
    bq_d = din("bq", [128, 1])
    bk_d = din("bk", [128, 1])
    gnsc_d = din("gnsc", [128, 2])
    gnbi_d = din("gnbi", [128, 2])
    sel_d = din("sel", [128, 64])
    rep_d = din("rep", [32, 256])
    augq_d = din("augq", [1, 2, HW], u8)        # fp8 bytes: 6.0 / 0.0
    augk_d = din("augk", [1, 2, HW], u8)        # fp8 bytes: 4.0 / 0.0
    out_d = nc.dram_tensor("outp", [128, 2, HW], f32, kind="ExternalOutput").ap()

    with tile.TileContext(nc) as tc, ExitStack() as ctx:
        persist = ctx.enter_context(tc.tile_pool(name="persist", bufs=1))

        # ---- long-lived tiles ----
        qTr = persist.tile([128, 2, HW], fp8, tag="qTr")   # parts 0-32 h0, 64-96 h1
        kTr = persist.tile([128, 2, HW], fp8, tag="kTr")
        v_aug = persist.tile([128, NJP, 2, 2, 128], fp8, tag="vaug")
        xn8 = persist.tile([128, 2, HW], fp8, tag="xn8")
        wqt = persist.tile([128, 2, 128], fp8, tag="wqt")
        wkt = persist.tile([128, 2, 128], fp8, tag="wkt")
        wvt = persist.tile([128, 2, 128], fp8, tag="wvt")
        nc.gpsimd.dma_start(wqt[:], wqb_d)
        nc.gpsimd.dma_start(wkt[:], wkb_d)
        nc.gpsimd.dma_start(wvt[:], wvb_d)
        wpf = persist.tile([64, 512], f32, tag="wpf")
        nc.gpsimd.dma_start(wpf[:], wp_d)
        wpr = persist.tile([64, 2, 2, 128], f32r, tag="wpr")
        nc.gpsimd.tensor_copy(wpr[:], wpf[:].rearrange("p (h o f) -> p h o f", h=2, o=2))
        bq = persist.tile([128, 1], f32, tag="bq")
        nc.gpsimd.dma_start(bq[:], bq_d)
        bk = persist.tile([128, 1], f32, tag="bk")
        nc.gpsimd.dma_start(bk[:], bk_d)
        expb = persist.tile([128, 1], f32, tag="expb")
        nc.vector.memset(expb[:], C_LN)
        onesf = persist.tile([65, 64], f32, tag="onesf")
        nc.gpsimd.memset(onesf[:], 1.0)
        ones64 = persist.tile([65, 64], f32r, tag="ones64")
        nc.gpsimd.tensor_copy(ones64[:], onesf[:])
        # ones column of v_aug
        nc.vector.memset(v_aug[:, :, :, :, 64:65], 1.0)

        # ---- load x + groupnorm stats ----
        with tc.tile_pool(name="early", bufs=1) as early, \
             tc.tile_pool(name="gnps", bufs=1, space="PSUM") as gnps:
            xt = early.tile([128, 2, HW], bf16, tag="xt")
            bnout = early.tile([128, 2, 8, 6], f32, tag="bnout")
            for c in (0, 1):
                for q in range(2):
                    nc.sync.dma_start(xt[:, c, ts(q, HW // 2)],
                                      x_d[c, :, ts(q, HW // 2)])
                    for u in range(4):
                        nc.vector.bn_stats(bnout[:, c, 4 * q + u, :],
                                           xt[:, c, ts(4 * q + u, 512)])
            # aug rows (constant fp8 bytes) into partitions 32 and 96;
            # emitted after the x loads so they don't delay the first chunk
            for base in (32, 96):
                nc.sync.dma_start(qTr[base:base + 1, :, :].bitcast(u8), augq_d)
                nc.sync.dma_start(kTr[base:base + 1, :, :].bitcast(u8), augk_d)
            gnsc = early.tile([128, 2], f32, tag="gnsc")
            gnbi = early.tile([128, 2], f32, tag="gnbi")
            nc.gpsimd.dma_start(gnsc[:], gnsc_d)
            nc.gpsimd.dma_start(gnbi[:], gnbi_d)
            sel = early.tile([128, 64], f32, tag="sel")
            nc.gpsimd.dma_start(sel[:], sel_d)
            rep = early.tile([32, 256], f32, tag="rep")
            nc.gpsimd.dma_start(rep[:], rep_d)

            stats = [early.tile([128, 2], f32, tag=f"st{c}", name=f"st{c}")
                     for c in (0, 1)]
            for c in (0, 1):
                nc.vector.bn_aggr(stats[c][:], bnout[:, c, :, :])  # (mean, var)
                mt = early.tile([128, 1], f32, tag="mt", name="mt")
                nc.vector.tensor_tensor(out=mt[:], in0=stats[c][:, 0:1],
                                        in1=stats[c][:, 0:1], op=ALU.mult)
                nc.vector.tensor_tensor(out=stats[c][:, 1:2],
                                        in0=stats[c][:, 1:2], in1=mt[:],
                                        op=ALU.add)
            gs_ps = gnps.tile([32, 2], f32, tag="gs")
            nc.tensor.matmul(gs_ps[:], lhsT=sel[:, 0:32], rhs=stats[0][:],
                             start=True, stop=False)
            nc.tensor.matmul(gs_ps[:], lhsT=sel[:, 32:64], rhs=stats[1][:],
                             start=False, stop=True)
            gs = early.tile([32, 2], f32, tag="gs_sb")
            nc.vector.tensor_copy(gs[:], gs_ps[:])
            rg = early.tile([32, 2], f32, tag="rg")  # col0 rstd, col1 mean
            msq = early.tile([32, 2], f32, tag="msq")
            nc.vector.tensor_copy(rg[:, 1:2], gs[:, 0:1])
            nc.vector.tensor_tensor(out=msq[:, 0:1], in0=gs[:, 0:1],
                                    in1=gs[:, 0:1], op=ALU.mult)
            nc.vector.tensor_tensor(out=msq[:, 1:2], in0=gs[:, 1:2],
                                    in1=msq[:, 0:1], op=ALU.subtract)
            eps_t = early.tile([32, 1], f32, tag="eps")
            nc.vector.memset(eps_t[:], EPS)
            sd = early.tile([32, 1], f32, tag="sd")
            nc.scalar.activation(sd[:], msq[:, 1:2], ACTF.Sqrt, bias=eps_t[:])
            nc.vector.reciprocal(rg[:, 0:1], sd[:])
            AB = [early.tile([128, 2], f32, tag=f"ab{c}", name=f"ab{c}")
                  for c in (0, 1)]
            for c in (0, 1):
                ab_ps = gnps.tile([128, 2], f32, tag="ab", name="ab")
                nc.tensor.matmul(ab_ps[:], lhsT=rep[:, ts(c, 128)], rhs=rg[:],
                                 start=True, stop=True)
                nc.vector.tensor_tensor(out=AB[c][:, 0:1], in0=ab_ps[:, 0:1],
                                        in1=gnsc[:, c:c + 1], op=ALU.mult)
                tmp = early.tile([128, 1], f32, tag=f"tmp{c}", name=f"tmp{c}")
                nc.vector.tensor_tensor(out=tmp[:], in0=ab_ps[:, 1:2],
                                        in1=AB[c][:, 0:1], op=ALU.mult)
                nc.vector.tensor_tensor(out=AB[c][:, 1:2], in0=gnbi[:, c:c + 1],
                                        in1=tmp[:], op=ALU.subtract)
            # xn = A*x + B -> fp8, split ACT/Pool/DVE
            for t in range(8):
                for c in (0, 1):
                    e = ("A", "P", "D")[(2 * t + c) % 3]
                    if e == "A":
                        nc.scalar.activation(xn8[:, c, ts(t, NI)],
                                             xt[:, c, ts(t, NI)],
                                             ACTF.Identity,
                                             scale=AB[c][:, 0:1],
                                             bias=AB[c][:, 1:2])
                    else:
                        en = nc.gpsimd if e == "P" else nc.vector
                        en.tensor_scalar(out=xn8[:, c, ts(t, NI)],
                                         in0=xt[:, c, ts(t, NI)],
                                         scalar1=AB[c][:, 0:1],
                                         scalar2=AB[c][:, 1:2],
                                         op0=ALU.mult, op1=ALU.add)

        # ---- QKV ----
        with tc.tile_pool(name="qkps", bufs=3, space="PSUM") as qkps, \
             tc.tile_pool(name="vps", bufs=2, space="PSUM") as vps, \
             tc.tile_pool(name="qkst", bufs=1) as qkst:
            k8 = qkst.tile([128, HW], fp8, tag="k8")
            q8 = qkst.tile([128, HW], fp8, tag="q8")

            def qk_chunk(w_t, b_t, dst, t, eng):
                ps = qkps.tile([128, NI], f32, tag="qk", name="qk")
                nc.tensor.matmul(ps[:], lhsT=w_t[:],
                                 rhs=xn8[:, :, ts(t, NI)],
                                 start=True, stop=True, perf_mode=DR)
                if eng == "D":
                    nc.vector.tensor_scalar(out=dst[:, ts(t, NI)],
                                            in0=ps[:], scalar1=b_t[:],
                                            scalar2=None, op0=ALU.add)
                else:
                    nc.scalar.activation(dst[:, ts(t, NI)], ps[:],
                                         ACTF.Identity, bias=b_t[:])

            def remap_chunk(src_t, dst_t, t):
                # partition regroup into DoubleRow layout for columns of chunk t
                for hh in (0, 1):
                    for sl in (0, 1):
                        nc.sync.dma_start(
                            dst_t[64 * hh:64 * hh + 32, sl, ts(t, NI)],
                            src_t[64 * hh + 32 * sl:64 * hh + 32 * sl + 32,
                                  ts(t, NI)])

            def v_chunk(jp, eng):
                vp = vps.tile([128, 2, 128], f32, tag="v", name="v")
                for jt in (0, 1):
                    nc.tensor.matmul(vp[:, jt, :],
                                     lhsT=xn8[:, :, ts(2 * jp + jt, 128)],
                                     rhs=wvt[:], start=True, stop=True,
                                     perf_mode=DR)
                # src order (jt, h*64+d) == dst order (jt, h, d)
                vv = vp[:].rearrange("p a (b c) -> p a b c", b=2)
                if eng == "D":
                    nc.vector.tensor_copy(v_aug[:, jp, :, :, 0:64], vv)
                else:
                    nc.scalar.activation(v_aug[:, jp, :, :, 0:64], vv,
                                         ACTF.Copy)

            for t in range(8):
                qk_chunk(wkt, bk, k8, t, "A" if t % 2 else "D")
            for t in range(8):
                qk_chunk(wqt, bq, q8, t, "D" if t % 2 else "A")
            for (s8, dst8) in ((k8, kTr), (q8, qTr)):
                for hh in (0, 1):
                    for sl in (0, 1):
                        en = nc.sync if (hh + sl) % 2 == 0 else nc.gpsimd
                        en.dma_start(
                            dst8[64 * hh:64 * hh + 32, sl, :],
                            s8[64 * hh + 32 * sl:64 * hh + 32 * sl + 32, :])
            for jp in range(NJP):
                v_chunk(jp, "A" if jp % 2 else "D")

        # ---- attention + normalize + projection ----
        # Flat pipelined emission: AV matmuls trail their exp by AV_LAG
        # chunks so the in-order PE never blocks score production on an
        # unfinished exp; proj for ic is emitted mid-way through ic+1.
        expb_i = 0
        with tc.tile_pool(name="spool", bufs=3, space="PSUM") as spool, \
             tc.tile_pool(name="avpool", bufs=2, space="PSUM") as avpool, \
             tc.tile_pool(name="ptpool", bufs=6) as ptpool, \
             tc.tile_pool(name="stpool", bufs=6) as stpool, \
             tc.tile_pool(name="stnpool", bufs=6) as stnpool, \
             tc.tile_pool(name="bcsp", bufs=3) as bcsp, \
             tc.tile_pool(name="ppst", bufs=3) as ppstp, \
             tc.tile_pool(name="sumsp", bufs=6) as sumsp:
            st_tiles = {}
            stn = {}
            pend = []          # (gchunk, (ic,h,jp), av, lhsT, pt, eng)

            def drain_one():
                _, (ic0, h0, jp0), av0, lhsT0, pt0, e0 = pend.pop(0)
                nc.tensor.matmul(av0[:], lhsT=lhsT0, rhs=pt0.bitcast(fp8),
                                 start=(jp0 == 0), stop=(jp0 == NJP - 1),
                                 perf_mode=DR)
                if jp0 == NJP - 1:
                    st = stpool.tile([65, NI], f32r, tag="st",
                                     name=f"st{ic0}{h0}")
                    if e0 == "A":
                        nc.scalar.activation(st[:], av0[:], ACTF.Copy)
                    else:
                        nc.vector.tensor_copy(st[:], av0[:])
                    st_tiles[(ic0, h0)] = st
                    sn = stnpool.tile([64, NI], f32r, tag="sn",
                                      name=f"sn{ic0}{h0}")
                    if ic0 == NIC - 1:
                        # tail path: no DMA hop; recip at partition 64,
                        # PE broadcast, DVE multiply
                        rr65 = stpool.tile([65, NI], f32r, tag="st",
                                           name=f"rr65{ic0}{h0}")
                        with nc.allow_low_precision(reason="recip rows"):
                            nc.vector.reciprocal(rr65[64:65, :],
                                                 st[64:65, :])
                        bcp = avpool.tile([64, NI], f32, tag="av",
                                          name=f"bcp{ic0}{h0}")
                        nc.tensor.matmul(bcp[:], lhsT=ones64[64:65, :],
                                         rhs=rr65[64:65, :],
                                         start=True, stop=True)
                        nc.vector.tensor_tensor(out=sn[:], in0=st[0:64, :],
                                                in1=bcp[:], op=ALU.mult)
                    else:
                        # recip row (DVE) -> broadcast + mult (Pool)
                        sr = sumsp.tile([1, NI], f32, tag="sr", name="sr")
                        nc.sync.dma_start(sr[:], st[64:65, :].bitcast(f32))
                        rr = sumsp.tile([1, NI], f32, tag="rr", name="rr")
                        nc.vector.reciprocal(rr[:], sr[:])
                        bcs = bcsp.tile([64, NI], f32, tag="bc", name="bc")
                        nc.gpsimd.partition_broadcast(bcs[:], rr[:])
                        nc.gpsimd.tensor_tensor(out=sn[:], in0=st[0:64, :],
                                                in1=bcs[:], op=ALU.mult)
                    stn[(ic0, h0)] = sn

            def emit_proj(ic0):
                pp = spool.tile([128, 2, NI], f32, tag="sp", name=f"pp{ic0}")
                for oc in (0, 1):
                    for h2 in (0, 1):
                        nc.tensor.matmul(pp[:, oc, :],
                                         lhsT=wpr[:, h2, oc, :],
                                         rhs=stn[(ic0, h2)][:],
                                         start=(h2 == 0), stop=(h2 == 1))
                po = ppstp.tile([128, 2, NI], f32, tag="po", name="po")
                nc.scalar.activation(po[:], pp[:], ACTF.Copy)
                nc.gpsimd.dma_start(out_d[:, :, ts(ic0, NI)], po[:])

            g = 0
            for ic in range(NIC):
                for h in (0, 1):
                    hb = 64 * h
                    av = avpool.tile([65, NI], f32, tag="av", name=f"av{ic}{h}")
                    for jp in range(NJP):
                        if h == 1 and jp == PROJ_JP and ic > 0:
                            emit_proj(ic - 1)
                        sp = spool.tile([128, 2, NI], f32, tag="sp", name="sp")
                        for jt in (0, 1):
                            nc.tensor.matmul(
                                sp[:, jt, :],
                                lhsT=kTr[hb:hb + 33, :, ts(2 * jp + jt, 128)],
                                rhs=qTr[hb:hb + 33, :, ts(ic, NI)],
                                start=True, stop=True, perf_mode=DR)
                        pt = ptpool.tile([128, 2, NI], u8, tag="pt", name="pt")
                        e = _EXP_PAT[expb_i % len(_EXP_PAT)]
                        expb_i += 1
                        if e == "A":
                            nc.scalar.activation(pt[:].bitcast(fp8), sp[:],
                                                 ACTF.Exp, scale=0.125,
                                                 bias=expb[:])
                        else:
                            nc.vector.tensor_scalar(out=pt[:], in0=sp[:],
                                                    scalar1=LN2_INV,
                                                    scalar2=0.0,
                                                    op0=ALU.mult, op1=ALU.max)
                        pend.append((g, (ic, h, jp), av,
                                     v_aug[:, jp, :, h, 0:65], pt[:], e))
                        while pend and pend[0][0] <= g - AV_LAG:
                            drain_one()
                        g += 1
            while pend:
                drain_one()
            emit_proj(NIC - 1)

    nc.compile()
    return nc


def _host_inputs(x, gn_scale, gn_bias, qkv_w, qkv_b, proj_w):
    import ml_dtypes
    x = np.ascontiguousarray(np.asarray(x, dtype=np.float32))
    gn_scale = np.asarray(gn_scale, dtype=np.float32)
    gn_bias = np.asarray(gn_bias, dtype=np.float32)
    qkv_w = np.asarray(qkv_w, dtype=np.float32)
    qkv_b = np.asarray(qkv_b, dtype=np.float32)
    proj_w = np.asarray(proj_w, dtype=np.float32)

    sel = np.zeros((128, 64), np.float32)
    rep = np.zeros((32, 256), np.float32)
    for p in range(128):
        sel[p, p // 8] = 1.0 / 8
        sel[p, 32 + 16 + p // 8] = 1.0 / 8
        rep[p // 8, p] = 1.0
        rep[16 + p // 8, 128 + p] = 1.0

    def aug_row(v):  # [1, 2, HW] fp8 bytes: slot0 = v, slot1 = 0
        a = np.zeros((1, 2, HW), dtype=ml_dtypes.float8_e4m3fn)
        a[0, 0, :] = v
        return np.ascontiguousarray(a.view(np.uint8))

    augq = aug_row(6.0)
    augk = aug_row(4.0)

    def wsel(W, rows):  # [256 c, sel 128 d2h] -> [128 cpart, 2 chalf, 128] fp8
        Wt = W[rows].T  # [256 c, 128]
        return np.ascontiguousarray(
            Wt.reshape(2, 128, 128).transpose(1, 0, 2)
            .astype(ml_dtypes.float8_e4m3fn))

    in_maps = []
    corrs = []
    for core in range(N_CORES):
        s, hg = core // 2, core % 2
        rows = np.r_[2 * hg * D:(2 * hg + 1) * D,
                     (2 * hg + 1) * D:(2 * hg + 2) * D]
        wq = wsel(qkv_w[0 * C:1 * C], rows)
        wk = wsel(qkv_w[1 * C:2 * C], rows)
        wv = wsel(qkv_w[2 * C:3 * C], rows)
        # wp[d, h, oc, :] = proj_w[oc*128:(oc+1)*128, rows[h*64+d]]
        wp = np.zeros((64, 2, 2, 128), np.float32)
        for h in (0, 1):
            block = proj_w[:, rows[h * 64:(h + 1) * 64]]  # [256 oc, 64 d]
            wp[:, h, 0, :] = block[0:128].T
            wp[:, h, 1, :] = block[128:256].T
        bqv = qkv_b[rows].reshape(128, 1)
        bkv = qkv_b[C + rows].reshape(128, 1)
        bv = qkv_b[2 * C + rows]
        corrs.append(proj_w[:, rows] @ bv)
        in_maps.append({
            "x_s": np.ascontiguousarray(x[s].reshape(2, 128, HW).astype(ml_dtypes.bfloat16)),
            "wqb": wq, "wkb": wk, "wvb": wv,
            "wp": np.ascontiguousarray(wp.reshape(64, 512)),
            "bq": np.ascontiguousarray(bqv), "bk": np.ascontiguousarray(bkv),
            "gnsc": np.ascontiguousarray(gn_scale.reshape(2, 128).T),
            "gnbi": np.ascontiguousarray(gn_bias.reshape(2, 128).T),
            "sel": sel, "rep": rep, "augq": augq, "augk": augk,
        })
    return x, in_maps, corrs


def kernel(x, gn_scale, gn_bias, qkv_w, qkv_b, proj_w, proj_b, _trace=False):
    from concourse import bass_utils

    if "nc" not in _cache:
        _cache["nc"] = _build_module()
    nc = _cache["nc"]

    x, in_maps, corrs = _host_inputs(x, gn_scale, gn_bias, qkv_w, qkv_b, proj_w)
    proj_b = np.asarray(proj_b, dtype=np.float32)

    res = bass_utils.run_bass_kernel_spmd(
        nc, in_maps, core_ids=list(range(N_CORES)), trace=_trace)
    _cache["last_result"] = res

    out = np.empty((B, C, Hs, Ws), np.float32)
    for s in range(B):
        acc = x[s].reshape(C, HW).copy()
        # outp [128 part, 2 oc, HW]: channel oc*128+p = outp[p, oc, :]
        for cr in (2 * s, 2 * s + 1):
            o = res.results[cr]["outp"]
            acc += o.transpose(1, 0, 2).reshape(C, HW)
        acc += (proj_b + corrs[2 * s] + corrs[2 * s + 1])[:, None]
        out[s] = acc.reshape(C, Hs, Ws)
    return out


# revision 51
# speedup vs baseline: 1.0366x; 1.0017x over previous
"""Trainium2 Bass kernel for nn_AttentionBlock (GroupNorm32 + 4-head self
attention over 64x64 spatial + output projection + residual).

Sharding over 8 NeuronCores: core = (sample s, head-group hg) with
s = core // 2, hg = core % 2 selecting global heads {2*hg, 2*hg+1}.

Per-core pipeline:
  groupnorm (bf16 x, f32 stats, fp8 xn) -> QKV via fp8 DoubleRow matmuls ->
  q,k bias-added to fp8e4 and DMA-remapped into DoubleRow layout [33, 2, HW]
  (d-halves stacked in the free dim; partition 32 carries a constant row
  contributing +24 to every score, which both shifts exp into a safe range
  and guards the fast-exp clamp) -> scores via fp8 DoubleRow matmuls
  (0.5 cyc/row, 2 k-tiles per instruction) -> exp alternating between ACT
  (native Exp -> fp8) and DVE (fast-exp bit trick rint(max(s'/ln2, 0)) ->
  uint8 == fp8e4 bytes); GPSIMD cannot touch PSUM so it instead handles the
  SBUF-side normalize (partition_broadcast of 1/sums + multiply) -> AV via
  fp8 DoubleRow with a trailing ones column producing softmax denominators
  in PSUM row 64 -> per-(ic,h) reciprocal on DVE (PE-broadcast fast path
  for the last i-chunk to shorten the tail) -> per-head f32r projection
  accumulated over heads in a spool PSUM slot -> [128, 2, HW] partial
  output per core.  AV matmuls trail their exp by AV_LAG chunks so the
  in-order PE never blocks score production on an unfinished exp; proj for
  ic is emitted mid-way through ic+1 so its stn dependency is long ready.
  Host adds residual, proj bias, and the constant v-bias correction
  proj_w[:,shard] @ bv.
"""
import numpy as np

NUM_GROUPS = 32
EPS = 1e-5
B, C, Hs, Ws = 4, 256, 64, 64
NHEADS = 4
D = C // NHEADS          # 64
HW = Hs * Ws             # 4096
N_CORES = 8
NI = 512                 # i-chunk (query positions per chunk)
NIC = HW // NI           # 8 i-chunks
NJP = HW // 256          # 16 j-tile-pairs per head
LN2_INV = float(1.0 / np.log(2.0))
AUG = 24.0               # constant added to every raw score via aug row
# ACT-path bias so both exp paths encode p = 2^{-56.5/8} * e^{s_raw/8}
C_LN = float(-AUG * 0.125 + (AUG / np.log(2.0) - 56.5) / 8.0 * np.log(2.0))

_cache = {}
AV_LAG = 3
PROJ_JP = NJP // 2

# exp-chunk engine schedule: ACT faster per row than DVE (0.833 vs 1.042)
_EXP_PAT = ("D", "A")


def _build_module():
    from contextlib import ExitStack
    import concourse.bass as bass
    import concourse.tile as tile
    from concourse import bacc, mybir

    f32 = mybir.dt.float32
    f32r = mybir.dt.float32r
    bf16 = mybir.dt.bfloat16
    fp8 = mybir.dt.float8e4
    u8 = mybir.dt.uint8
    ALU = mybir.AluOpType
    ACTF = mybir.ActivationFunctionType
    DR = mybir.MatmulPerfMode.DoubleRow
    ts = bass.ts

    nc = bacc.Bacc("TRN2", target_bir_lowering=False, debug=False,
                   num_devices=N_CORES)

    def din(name, shape, dt=f32):
        return nc.dram_tensor(name, shape, dt, kind="ExternalInput").ap()

    x_d = din("x_s", [2, 128, HW], bf16)        # c-half, c-part, i
    wqb_d = din("wqb", [128, 2, 128], fp8)      # c-part, c-half, d2h
    wkb_d = din("wkb", [128, 2, 128], fp8)
    wvb_d = din("wvb", [128, 2, 128], fp8)
    wp_d = din("wp", [64, 512])                 # d, (h, oc)*128  (f32)
    bq_d = din("bq", [128, 1])
    bk_d = din("bk", [128, 1])
    gnsc_d = din("gnsc", [128, 2])
    gnbi_d = din("gnbi", [128, 2])
    sel_d = din("sel", [128, 64])
    rep_d = din("rep", [32, 256])
    augq_d = din("augq", [1, 2, HW], u8)        # fp8 bytes: 6.0 / 0.0
    augk_d = din("augk", [1, 2, HW], u8)        # fp8 bytes: 4.0 / 0.0
    out_d = nc.dram_tensor("outp", [128, 2, HW], f32, kind="ExternalOutput").ap()

    with tile.TileContext(nc) as tc, ExitStack() as ctx:
        persist = ctx.enter_context(tc.tile_pool(name="persist", bufs=1))

        # ---- long-lived tiles ----
        qTr = persist.tile([128, 2, HW], fp8, tag="qTr")   # parts 0-32 h0, 64-96 h1
        kTr = persist.tile([128, 2, HW], fp8, tag="kTr")
        v_aug = persist.tile([128, NJP, 2, 2, 128], fp8, tag="vaug")
        xn8 = persist.tile([128, 2, HW], fp8, tag="xn8")
        wqt = persist.tile([128, 2, 128], fp8, tag="wqt")
        wkt = persist.tile([128, 2, 128], fp8, tag="wkt")
        wvt = persist.tile([128, 2, 128], fp8, tag="wvt")
        nc.gpsimd.dma_start(wqt[:], wqb_d)
        nc.gpsimd.dma_start(wkt[:], wkb_d)
        nc.gpsimd.dma_start(wvt[:], wvb_d)
        wpf = persist.tile([64, 512], f32, tag="wpf")
        nc.gpsimd.dma_start(wpf[:], wp_d)
        wpr = persist.tile([64, 2, 2, 128], f32r, tag="wpr")
        nc.gpsimd.tensor_copy(wpr[:], wpf[:].rearrange("p (h o f) -> p h o f", h=2, o=2))
        bq = persist.tile([128, 1], f32, tag="bq")
        nc.gpsimd.dma_start(bq[:], bq_d)
        bk = persist.tile([128, 1], f32, tag="bk")
        nc.gpsimd.dma_start(bk[:], bk_d)
        expb = persist.tile([128, 1], f32, tag="expb")
        nc.vector.memset(expb[:], C_LN)
        onesf = persist.tile([65, 64], f32, tag="onesf")
        nc.gpsimd.memset(onesf[:], 1.0)
        ones64 = persist.tile([65, 64], f32r, tag="ones64")
        nc.gpsimd.tensor_copy(ones64[:], onesf[:])
        # ones column of v_aug
        nc.vector.memset(v_aug[:, :, :, :, 64:65], 1.0)

        # ---- load x + groupnorm stats ----
        with tc.tile_pool(name="early", bufs=1) as early, \
             tc.tile_pool(name="gnps", bufs=1, space="PSUM") as gnps:
            xt = early.tile([128, 2, HW], bf16, tag="xt")
            bnout = early.tile([128, 2, 8, 6], f32, tag="bnout")
            for c in (0, 1):
                for q in range(2):
                    nc.sync.dma_start(xt[:, c, ts(q, HW // 2)],
                                      x_d[c, :, ts(q, HW // 2)])
                    for u in range(4):
                        nc.vector.bn_stats(bnout[:, c, 4 * q + u, :],
                                           xt[:, c, ts(4 * q + u, 512)])
            # aug rows (constant fp8 bytes) into partitions 32 and 96;
            # emitted after the x loads so they don't delay the first chunk
            for base in (32, 96):
                nc.sync.dma_start(qTr[base:base + 1, :, :].bitcast(u8), augq_d)
                nc.sync.dma_start(kTr[base:base + 1, :, :].bitcast(u8), augk_d)
            gnsc = early.tile([128, 2], f32, tag="gnsc")
            gnbi = early.tile([128, 2], f32, tag="gnbi")
            nc.sync.dma_start(gnsc[:], gnsc_d)
            nc.sync.dma_start(gnbi[:], gnbi_d)
            sel = early.tile([128, 64], f32, tag="sel")
            nc.sync.dma_start(sel[:], sel_d)
            rep = early.tile([32, 256], f32, tag="rep")
            nc.sync.dma_start(rep[:], rep_d)

            stats = [early.tile([128, 2], f32, tag=f"st{c}", name=f"st{c}")
                     for c in (0, 1)]
            for c in (0, 1):
                nc.vector.bn_aggr(stats[c][:], bnout[:, c, :, :])  # (mean, var)
                mt = early.tile([128, 1], f32, tag="mt", name="mt")
                nc.vector.tensor_tensor(out=mt[:], in0=stats[c][:, 0:1],
                                        in1=stats[c][:, 0:1], op=ALU.mult)
                nc.vector.tensor_tensor(out=stats[c][:, 1:2],
                                        in0=stats[c][:, 1:2], in1=mt[:],
                                        op=ALU.add)
            gs_ps = gnps.tile([32, 2], f32, tag="gs")
            nc.tensor.matmul(gs_ps[:], lhsT=sel[:, 0:32], rhs=stats[0][:],
                             start=True, stop=False)
            nc.tensor.matmul(gs_ps[:], lhsT=sel[:, 32:64], rhs=stats[1][:],
                             start=False, stop=True)
            gs = early.tile([32, 2], f32, tag="gs_sb")
            nc.vector.tensor_copy(gs[:], gs_ps[:])
            rg = early.tile([32, 2], f32, tag="rg")  # col0 rstd, col1 mean
            msq = early.tile([32, 2], f32, tag="msq")
            nc.vector.tensor_copy(rg[:, 1:2], gs[:, 0:1])
            nc.vector.tensor_tensor(out=msq[:, 0:1], in0=gs[:, 0:1],
                                    in1=gs[:, 0:1], op=ALU.mult)
            nc.vector.tensor_tensor(out=msq[:, 1:2], in0=gs[:, 1:2],
                                    in1=msq[:, 0:1], op=ALU.subtract)
            eps_t = early.tile([32, 1], f32, tag="eps")
            nc.vector.memset(eps_t[:], EPS)
            sd = early.tile([32, 1], f32, tag="sd")
            nc.scalar.activation(sd[:], msq[:, 1:2], ACTF.Sqrt, bias=eps_t[:])
            nc.vector.reciprocal(rg[:, 0:1], sd[:])
            AB = [early.tile([128, 2], f32, tag=f"ab{c}", name=f"ab{c}")
                  for c in (0, 1)]
            for c in (0, 1):
                ab_ps = gnps.tile([128, 2], f32, tag="ab", name="ab")
                nc.tensor.matmul(ab_ps[:], lhsT=rep[:, ts(c, 128)], rhs=rg[:],
                                 start=True, stop=True)
                nc.vector.tensor_tensor(out=AB[c][:, 0:1], in0=ab_ps[:, 0:1],
                                        in1=gnsc[:, c:c + 1], op=ALU.mult)
                tmp = early.tile([128, 1], f32, tag=f"tmp{c}", name=f"tmp{c}")
                nc.vector.tensor_tensor(out=tmp[:], in0=ab_ps[:, 1:2],
                                        in1=AB[c][:, 0:1], op=ALU.mult)
                nc.vector.tensor_tensor(out=AB[c][:, 1:2], in0=gnbi[:, c:c + 1],
                                        in1=tmp[:], op=ALU.subtract)
            # xn = A*x + B -> fp8, split ACT/Pool/DVE
            for t in range(8):
                for c in (0, 1):
                    e = ("A", "P", "D")[(2 * t + c) % 3]
                    if e == "A":
                        nc.scalar.activation(xn8[:, c, ts(t, NI)],
                                             xt[:, c, ts(t, NI)],
                                             ACTF.Identity,
                                             scale=AB[c][:, 0:1],
                                             bias=AB[c][:, 1:2])
                    else:
                        en = nc.gpsimd if e == "P" else nc.vector
                        en.tensor_scalar(out=xn8[:, c, ts(t, NI)],
                                         in0=xt[:, c, ts(t, NI)],
                                         scalar1=AB[c][:, 0:1],
                                         scalar2=AB[c][:, 1:2],
                                         op0=ALU.mult, op1=ALU.add)

        # ---- QKV ----
        with tc.tile_pool(name="qkps", bufs=3, space="PSUM") as qkps, \
             tc.tile_pool(name="vps", bufs=2, space="PSUM") as vps, \
             tc.tile_pool(name="qkst", bufs=1) as qkst:
            k8 = qkst.tile([128, HW], fp8, tag="k8")
            q8 = qkst.tile([128, HW], fp8, tag="q8")

            def qk_chunk(w_t, b_t, dst, t, eng):
                ps = qkps.tile([128, NI], f32, tag="qk", name="qk")
                nc.tensor.matmul(ps[:], lhsT=w_t[:],
                                 rhs=xn8[:, :, ts(t, NI)],
                                 start=True, stop=True, perf_mode=DR)
                if eng == "D":
                    nc.vector.tensor_scalar(out=dst[:, ts(t, NI)],
                                            in0=ps[:], scalar1=b_t[:],
                                            scalar2=None, op0=ALU.add)
                else:
                    nc.scalar.activation(dst[:, ts(t, NI)], ps[:],
                                         ACTF.Identity, bias=b_t[:])

            def remap_chunk(src_t, dst_t, t):
                # partition regroup into DoubleRow layout for columns of chunk t
                for hh in (0, 1):
                    for sl in (0, 1):
                        nc.sync.dma_start(
                            dst_t[64 * hh:64 * hh + 32, sl, ts(t, NI)],
                            src_t[64 * hh + 32 * sl:64 * hh + 32 * sl + 32,
                                  ts(t, NI)])

            def v_chunk(jp, eng):
                vp = vps.tile([128, 2, 128], f32, tag="v", name="v")
                for jt in (0, 1):
                    nc.tensor.matmul(vp[:, jt, :],
                                     lhsT=xn8[:, :, ts(2 * jp + jt, 128)],
                                     rhs=wvt[:], start=True, stop=True,
                                     perf_mode=DR)
                # src order (jt, h*64+d) == dst order (jt, h, d)
                vv = vp[:].rearrange("p a (b c) -> p a b c", b=2)
                if eng == "D":
                    nc.vector.tensor_copy(v_aug[:, jp, :, :, 0:64], vv)
                else:
                    nc.scalar.activation(v_aug[:, jp, :, :, 0:64], vv,
                                         ACTF.Copy)

            for t in range(8):
                qk_chunk(wkt, bk, k8, t, "A" if t % 2 else "D")
            for t in range(8):
                qk_chunk(wqt, bq, q8, t, "D" if t % 2 else "A")
            for (s8, dst8) in ((k8, kTr), (q8, qTr)):
                for hh in (0, 1):
                    for sl in (0, 1):
                        en = nc.sync if (hh + sl) % 2 == 0 else nc.gpsimd
                        en.dma_start(
                            dst8[64 * hh:64 * hh + 32, sl, :],
                            s8[64 * hh + 32 * sl:64 * hh + 32 * sl + 32, :])
            for jp in range(NJP):
                v_chunk(jp, "A" if jp % 2 else "D")

        # ---- attention + normalize + projection ----
        # Flat pipelined emission: AV matmuls trail their exp by AV_LAG
        # chunks so the in-order PE never blocks score production on an
        # unfinished exp; proj for ic is emitted mid-way through ic+1.
        expb_i = 0
        with tc.tile_pool(name="spool", bufs=3, space="PSUM") as spool, \
             tc.tile_pool(name="avpool", bufs=2, space="PSUM") as avpool, \
             tc.tile_pool(name="ptpool", bufs=6) as ptpool, \
             tc.tile_pool(name="stpool", bufs=6) as stpool, \
             tc.tile_pool(name="stnpool", bufs=6) as stnpool, \
             tc.tile_pool(name="bcsp", bufs=3) as bcsp, \
             tc.tile_pool(name="ppst", bufs=3) as ppstp, \
             tc.tile_pool(name="sumsp", bufs=6) as sumsp:
            st_tiles = {}
            stn = {}
            pend = []          # (gchunk, (ic,h,jp), av, lhsT, pt, eng)

            def drain_one():
                _, (ic0, h0, jp0), av0, lhsT0, pt0, e0 = pend.pop(0)
                nc.tensor.matmul(av0[:], lhsT=lhsT0, rhs=pt0.bitcast(fp8),
                                 start=(jp0 == 0), stop=(jp0 == NJP - 1),
                                 perf_mode=DR)
                if jp0 == NJP - 1:
                    st = stpool.tile([65, NI], f32r, tag="st",
                                     name=f"st{ic0}{h0}")
                    if e0 == "A":
                        nc.scalar.activation(st[:], av0[:], ACTF.Copy)
                    else:
                        nc.vector.tensor_copy(st[:], av0[:])
                    st_tiles[(ic0, h0)] = st
                    sn = stnpool.tile([64, NI], f32r, tag="sn",
                                      name=f"sn{ic0}{h0}")
                    if ic0 == NIC - 1:
                        # tail path: no DMA hop; recip at partition 64,
                        # PE broadcast, DVE multiply
                        rr65 = stpool.tile([65, NI], f32r, tag="st",
                                           name=f"rr65{ic0}{h0}")
                        with nc.allow_low_precision(reason="recip rows"):
                            nc.vector.reciprocal(rr65[64:65, :],
                                                 st[64:65, :])
                        bcp = avpool.tile([64, NI], f32, tag="av",
                                          name=f"bcp{ic0}{h0}")
                        nc.tensor.matmul(bcp[:], lhsT=ones64[64:65, :],
                                         rhs=rr65[64:65, :],
                                         start=True, stop=True)
                        nc.vector.tensor_tensor(out=sn[:], in0=st[0:64, :],
                                                in1=bcp[:], op=ALU.mult)
                    else:
                        # recip row (DVE) -> broadcast + mult (Pool)
                        sr = sumsp.tile([1, NI], f32, tag="sr", name="sr")
                        nc.sync.dma_start(sr[:], st[64:65, :].bitcast(f32))
                        rr = sumsp.tile([1, NI], f32, tag="rr", name="rr")
                        nc.vector.reciprocal(rr[:], sr[:])
                        bcs = bcsp.tile([64, NI], f32, tag="bc", name="bc")
                        nc.gpsimd.partition_broadcast(bcs[:], rr[:])
                        nc.gpsimd.tensor_tensor(out=sn[:], in0=st[0:64, :],
                                                in1=bcs[:], op=ALU.mult)
                    stn[(ic0, h0)] = sn

            def emit_proj(ic0):
                pp = spool.tile([128, 2, NI], f32, tag="sp", name=f"pp{ic0}")
                for oc in (0, 1):
                    for h2 in (0, 1):
                        nc.tensor.matmul(pp[:, oc, :],
                                         lhsT=wpr[:, h2, oc, :],
                                         rhs=stn[(ic0, h2)][:],
                                         start=(h2 == 0), stop=(h2 == 1))
                po = ppstp.tile([128, 2, NI], f32, tag="po", name="po")
                nc.scalar.activation(po[:], pp[:], ACTF.Copy)
                nc.gpsimd.dma_start(out_d[:, :, ts(ic0, NI)], po[:])

            g = 0
            for ic in range(NIC):
                for h in (0, 1):
                    hb = 64 * h
                    av = avpool.tile([65, NI], f32, tag="av", name=f"av{ic}{h}")
                    for jp in range(NJP):
                        if h == 1 and jp == PROJ_JP and ic > 0:
                            emit_proj(ic - 1)
                        sp = spool.tile([128, 2, NI], f32, tag="sp", name="sp")
                        for jt in (0, 1):
                            nc.tensor.matmul(
                                sp[:, jt, :],
                                lhsT=kTr[hb:hb + 33, :, ts(2 * jp + jt, 128)],
                                rhs=qTr[hb:hb + 33, :, ts(ic, NI)],
                                start=True, stop=True, perf_mode=DR)
                        pt = ptpool.tile([128, 2, NI], u8, tag="pt", name="pt")
                        e = _EXP_PAT[expb_i % len(_EXP_PAT)]
                        expb_i += 1
                        if e == "A":
                            nc.scalar.activation(pt[:].bitcast(fp8), sp[:],
                                                 ACTF.Exp, scale=0.125,
                                                 bias=expb[:])
                        else:
                            nc.vector.tensor_scalar(out=pt[:], in0=sp[:],
                                                    scalar1=LN2_INV,
                                                    scalar2=0.0,
                                                    op0=ALU.mult, op1=ALU.max)
                        pend.append((g, (ic, h, jp), av,
                                     v_aug[:, jp, :, h, 0:65], pt[:], e))
                        while pend and pend[0][0] <= g - AV_LAG:
                            drain_one()
                        g += 1
            while pend:
                drain_one()
            emit_proj(NIC - 1)

    nc.compile()
    return nc


def _host_inputs(x, gn_scale, gn_bias, qkv_w, qkv_b, proj_w):
    import ml_dtypes
    x = np.ascontiguousarray(np.asarray(x, dtype=np.float32))
    gn_scale = np.asarray(gn_scale, dtype=np.float32)
    gn_bias = np.asarray(gn_bias, dtype=np.float32)
    qkv_w = np.asarray(qkv_w, dtype=np.float32)
    qkv_b = np.asarray(qkv_b, dtype=np.float32)
    proj_w = np.asarray(proj_w, dtype=np.float32)

    sel = np.zeros((128, 64), np.float32)
    rep = np.zeros((32, 256), np.float32)
    for p in range(128):
        sel[p, p // 8] = 1.0 / 8
        sel[p, 32 + 16 + p // 8] = 1.0 / 8
        rep[p // 8, p] = 1.0
        rep[16 + p // 8, 128 + p] = 1.0

    def aug_row(v):  # [1, 2, HW] fp8 bytes: slot0 = v, slot1 = 0
        a = np.zeros((1, 2, HW), dtype=ml_dtypes.float8_e4m3fn)
        a[0, 0, :] = v
        return np.ascontiguousarray(a.view(np.uint8))

    augq = aug_row(6.0)
    augk = aug_row(4.0)

    def wsel(W, rows):  # [256 c, sel 128 d2h] -> [128 cpart, 2 chalf, 128] fp8
        Wt = W[rows].T  # [256 c, 128]
        return np.ascontiguousarray(
            Wt.reshape(2, 128, 128).transpose(1, 0, 2)
            .astype(ml_dtypes.float8_e4m3fn))

    in_maps = []
    corrs = []
    for core in range(N_CORES):
        s, hg = core // 2, core % 2
        rows = np.r_[2 * hg * D:(2 * hg + 1) * D,
                     (2 * hg + 1) * D:(2 * hg + 2) * D]
        wq = wsel(qkv_w[0 * C:1 * C], rows)
        wk = wsel(qkv_w[1 * C:2 * C], rows)
        wv = wsel(qkv_w[2 * C:3 * C], rows)
        # wp[d, h, oc, :] = proj_w[oc*128:(oc+1)*128, rows[h*64+d]]
        wp = np.zeros((64, 2, 2, 128), np.float32)
        for h in (0, 1):
            block = proj_w[:, rows[h * 64:(h + 1) * 64]]  # [256 oc, 64 d]
            wp[:, h, 0, :] = block[0:128].T
            wp[:, h, 1, :] = block[128:256].T
        bqv = qkv_b[rows].reshape(128, 1)
        bkv = qkv_b[C + rows].reshape(128, 1)
        bv = qkv_b[2 * C + rows]
        corrs.append(proj_w[:, rows] @ bv)
        in_maps.append({
            "x_s": np.ascontiguousarray(x[s].reshape(2, 128, HW).astype(ml_dtypes.bfloat16)),
            "wqb": wq, "wkb": wk, "wvb": wv,
            "wp": np.ascontiguousarray(wp.reshape(64, 512)),
            "bq": np.ascontiguousarray(bqv), "bk": np.ascontiguousarray(bkv),
            "gnsc": np.ascontiguousarray(gn_scale.reshape(2, 128).T),
            "gnbi": np.ascontiguousarray(gn_bias.reshape(2, 128).T),
            "sel": sel, "rep": rep, "augq": augq, "augk": augk,
        })
    return x, in_maps, corrs


def kernel(x, gn_scale, gn_bias, qkv_w, qkv_b, proj_w, proj_b, _trace=False):
    from concourse import bass_utils

    if "nc" not in _cache:
        _cache["nc"] = _build_module()
    nc = _cache["nc"]

    x, in_maps, corrs = _host_inputs(x, gn_scale, gn_bias, qkv_w, qkv_b, proj_w)
    proj_b = np.asarray(proj_b, dtype=np.float32)

    res = bass_utils.run_bass_kernel_spmd(
        nc, in_maps, core_ids=list(range(N_CORES)), trace=_trace)
    _cache["last_result"] = res

    out = np.empty((B, C, Hs, Ws), np.float32)
    for s in range(B):
        acc = x[s].reshape(C, HW).copy()
        # outp [128 part, 2 oc, HW]: channel oc*128+p = outp[p, oc, :]
        for cr in (2 * s, 2 * s + 1):
            o = res.results[cr]["outp"]
            acc += o.transpose(1, 0, 2).reshape(C, HW)
        acc += (proj_b + corrs[2 * s] + corrs[2 * s + 1])[:, None]
        out[s] = acc.reshape(C, Hs, Ws)
    return out
